# revision 1
# baseline (speedup 1.0000x reference)
"""GATv2 attention-weights kernel for 8 Trainium2 NeuronCores.

Problem (per full input):
    q: (2, 8, 384, 64) f32, k: (2, 8, 384, 64) f32,
    attention: (1, 8, 1, 1, 64) f32, mask: (2, 8, 384, 384) bool
    scores[b,h,i,j] = sum_d silu(q[b,h,i,d] + k[b,h,j,d]) * attention[h,d]
    out = softmax over j with mask (-inf before, 0 after)

Sharding: data-parallel over the 16 (b,h) pairs, 2 per core.

Per-core device pipeline (raw bass, explicit semaphores; "jj,d" packing =
two j columns share the 128 partitions, d=64 each half):
    - DVE builds T[(jj,d), i] = qT_rep + k_pair  (per-partition-scalar add,
      2x perf mode) for G j-pairs per group
    - ACT computes silu IN PLACE on T (ACT is the throughput floor:
      LQ*LK*D silu evaluations per (b,h) at 128 lanes / 1.2 GHz)
    - PE reduces over d with the `a` vector folded into the weights:
      matmul(lhsT=T_block[(jj,d), i_block], rhs=a2[(jj,d), 2]) ->
      scores[i_block, j_pair] land un-transposed in PSUM (6 banks hold all
      scores for both (b,h))
    - Masked softmax over the free dim afterwards (one activation-table
      switch to Exp for the whole kernel): fused (mask*-1e30)+scores on DVE,
      exp with fused row-sum (accum_out) on ACT, reciprocal + scale on DVE.
      No per-row max: scores are bounded (|s| < 8), exp cannot overflow.
"""

import numpy as np
from contextlib import ExitStack

import concourse.bass as bass
from concourse import mybir
from concourse.bass_utils import run_bass_kernel_spmd

B, H, LQ, LK, D = 2, 8, 384, 384, 64
NCORES = 8
NBH = (B * H) // NCORES        # 2 (b,h) pairs per core
NPAIR = LK // 2                # 192 j-pairs
# j-pairs per silu group: bh0 ramps up so the pipeline fills fast, then
# steady-state groups are large to amortize the ACT per-instruction overhead
GROUPS_BH = [[2, 4, 8, 8, 12, 16, 20, 26, 32, 36, 28], [36, 36, 36, 36, 36, 12]]
assert all(sum(g) == NPAIR for g in GROUPS_BH)
GMAX = max(max(g) for g in GROUPS_BH)
# flattened per-rep schedule: (bh, size, pair_offset)
GLIST = [(bh, s, off)
         for bh in range(2)
         for s, off in zip(GROUPS_BH[bh],
                           [sum(GROUPS_BH[bh][:i])
                            for i in range(len(GROUPS_BH[bh]))])]
G0 = len(GROUPS_BH[0])         # groups in bh0
GG = len(GLIST)                # global groups per rep
NIB = LQ // 128                # 3 i-blocks
NSM = NBH * NIB                # 6 softmax tiles
QKA = LQ + NPAIR + 2           # packed constants width (per partition, f32)

_f32 = mybir.dt.float32
_u8 = mybir.dt.uint8

_built = None  # cache across calls


def _build(reps=1):
    # reps > 1 unrolls the whole computation N times inside one program
    # (used only for steady-state timing; the grading path uses reps=1).
    AF = mybir.ActivationFunctionType
    Alu = mybir.AluOpType

    nc = bass.Bass("TRN2", target_bir_lowering=False, debug=False,
                   num_devices=NCORES)

    qka_d = nc.dram_tensor("qka", [NBH, 128, QKA], _f32, kind="ExternalInput").ap()
    mask_d = nc.dram_tensor("masku8", [NBH, LQ, LK], _u8, kind="ExternalInput").ap()
    w_d = nc.dram_tensor("w", [NBH, LQ, LK], _f32, kind="ExternalOutput").ap()

    qka_t = [nc.alloc_sbuf_tensor(f"qka_t{bh}", [128, QKA], _f32).ap()
             for bh in range(NBH)]
    mask_t = [nc.alloc_sbuf_tensor(f"mask_t{i}", [128, LK], _u8).ap()
              for i in range(NSM)]
    T_t = [nc.alloc_sbuf_tensor(f"T{s}", [128, GMAX * LQ], _f32).ap()
           for s in range(3)]
    E_t = [nc.alloc_sbuf_tensor(f"E{i}", [128, LK], _f32).ap()
           for i in range(NSM)]
    W_t = [nc.alloc_sbuf_tensor(f"W{i}", [128, LK], _f32).ap()
           for i in range(NSM)]
    sums_t = [nc.alloc_sbuf_tensor(f"sums{i}", [128, 1], _f32).ap()
              for i in range(NSM)]
    r_t = [nc.alloc_sbuf_tensor(f"r{i}", [128, 1], _f32).ap()
           for i in range(NSM)]
    sc_t = [nc.alloc_psum_tensor(f"sc{i}", [128, LK], _f32).ap()
            for i in range(NSM)]

    def qtrep(bh):
        return qka_t[bh][:, 0:LQ]

    def kpair(bh, p):
        return qka_t[bh][:, LQ + p:LQ + p + 1]

    def a2(bh):
        return qka_t[bh][:, LQ + NPAIR:LQ + NPAIR + 2]

    with ExitStack() as ctx:
        s_qka = [ctx.enter_context(nc.semaphore(f"s_qka{bh}")) for bh in range(NBH)]
        s_qk0b = ctx.enter_context(nc.semaphore("s_qk0b"))
        s_qk0c = ctx.enter_context(nc.semaphore("s_qk0c"))
        s_mask = ctx.enter_context(nc.semaphore("s_mask"))
        s_T = ctx.enter_context(nc.semaphore("s_T"))
        s_S = ctx.enter_context(nc.semaphore("s_S"))
        s_pe = ctx.enter_context(nc.semaphore("s_pe"))
        s_scm = ctx.enter_context(nc.semaphore("s_scm"))
        s_E = ctx.enter_context(nc.semaphore("s_E"))
        s_W = ctx.enter_context(nc.semaphore("s_W"))
        s_wsp = ctx.enter_context(nc.semaphore("s_wsp"))
        s_wact = ctx.enter_context(nc.semaphore("s_wact"))
        block = ctx.enter_context(nc.Block())

        CH0 = LQ + 16  # first chunk: qtrep + 16 kpairs (gates group 0..2)

        @block.sync
        def _(sp):
            # bh0 leading chunk first (everything upstream gates on it),
            # a2 rides in the same chunk via a second tiny DMA on the queue
            sp.dma_start(out=qka_t[0][:, 0:CH0],
                         in_=qka_d[0][:, 0:CH0]).then_inc(s_qka[0], 16)
            sp.dma_start(out=qka_t[0][:, LQ + NPAIR:LQ + NPAIR + 2],
                         in_=qka_d[0][:, LQ + NPAIR:LQ + NPAIR + 2]
                         ).then_inc(s_qk0b, 16)
            sp.dma_start(out=qka_t[0][:, CH0:LQ + NPAIR],
                         in_=qka_d[0][:, CH0:LQ + NPAIR]
                         ).then_inc(s_qk0c, 16)
            sp.dma_start(out=qka_t[1], in_=qka_d[1]).then_inc(s_qka[1], 16)
            for idx in range(NSM):
                bh, ib = divmod(idx, NIB)
                sp.dma_start(out=mask_t[idx],
                             in_=mask_d[bh, ib * 128:(ib + 1) * 128, :]
                             ).then_inc(s_mask, 16)
            # output DMAs: late tiles (3,4,5) on the SP HWDGE queue,
            # early tiles (0,1,2) on ACT's HWDGE queue (see scalar block)
            for rep in range(reps):
                for idx in range(NSM // 2, NSM):
                    bh, ib = divmod(idx, NIB)
                    sp.wait_ge(s_W, rep * NSM + idx + 1)
                    sp.dma_start(out=w_d[bh, ib * 128:(ib + 1) * 128, :],
                                 in_=W_t[idx]).then_inc(s_wsp, 16)
            sp.wait_ge(s_wsp, 16 * (NSM // 2) * reps)
            sp.wait_ge(s_wact, 16 * (NSM // 2) * reps)

        @block.vector
        def _(v):
            def tbuild(v, rep, gg):
                bh, size, off = GLIST[gg]
                gi = rep * GG + gg
                if rep == 0 and bh == 1 and off == 0:
                    v.wait_ge(s_qka[1], 16)
                if gi >= 3:
                    v.wait_ge(s_pe, gi - 2)
                T = T_t[gi % 3]
                for pl in range(size):
                    p = off + pl
                    ins = v.tensor_scalar_add(
                        T[:, pl * LQ:(pl + 1) * LQ], qtrep(bh), kpair(bh, p))
                ins.then_inc(s_T, 1)

            def scm(v, rep, bh):
                # mask+scores fuse for this bh.  bh0's is emitted a few
                # groups into bh1's stream so its s_pe wait is already
                # implied and DVE never stalls at the bh boundary.
                if rep == 0 and bh == 0:
                    v.wait_ge(s_mask, 16 * NSM)
                for ib in range(NIB):
                    idx = bh * NIB + ib
                    v.wait_ge(s_pe, rep * GG + (G0 if bh == 0 else GG))
                    if rep >= 1:
                        # scm tile reuse: previous rep's exp must be done
                        v.wait_ge(s_E, (rep - 1) * NSM + idx + 1)
                    v.scalar_tensor_tensor(
                        sc_t[idx], mask_t[idx], -1e30, sc_t[idx],
                        Alu.mult, Alu.add).then_inc(s_scm, 1)

            v.wait_ge(s_qka[0], 16)
            for rep in range(reps):
                for gg in range(GG):
                    bh_, size_, off_ = GLIST[gg]
                    if rep == 0 and bh_ == 0 and off_ < 16 <= off_ + size_:
                        v.wait_ge(s_qk0c, 16)
                    tbuild(v, rep, gg)
                    if gg == G0 + 2:
                        scm(v, rep, 0)  # bh0 softmax prep, overlapped
                scm(v, rep, 1)
                for idx in range(NSM):
                    v.wait_ge(s_E, rep * NSM + idx + 1)
                    if rep >= 1:
                        # W tile reuse: all of the previous rep's output DMAs
                        # on the owning queue must be done (conservative --
                        # cross-DMA order within a queue isn't assumed)
                        qs = s_wact if idx < NSM // 2 else s_wsp
                        v.wait_ge(qs, 16 * (NSM // 2) * rep)
                    v.reciprocal(r_t[idx], sums_t[idx])
                    v.drain()  # r is a scalar operand of the next op
                    v.tensor_scalar_mul(W_t[idx], E_t[idx],
                                        r_t[idx]).then_inc(s_W, 1)

        @block.scalar
        def _(a):
            for rep in range(reps):
                for gg in range(GG):
                    _, size, _ = GLIST[gg]
                    a.wait_ge(s_T, rep * GG + gg + 1)
                    T = T_t[(rep * GG + gg) % 3]
                    a.activation(T[:, 0:size * LQ], T[:, 0:size * LQ],
                                 AF.Silu).then_inc(s_S, 1)
                for idx in range(NSM):
                    a.wait_ge(s_scm, rep * NSM + idx + 1)
                    if rep >= 1:
                        # E/sums tile reuse: previous rep's W-scale must be done
                        a.wait_ge(s_W, (rep - 1) * NSM + idx + 1)
                    a.activation(E_t[idx], sc_t[idx], AF.Exp,
                                 accum_out=sums_t[idx]).then_inc(s_E, 1)
                # early output tiles on ACT's HWDGE queue (waits are
                # already satisfied by the time the last exp retires)
                for idx in range(NSM // 2):
                    bh, ib = divmod(idx, NIB)
                    a.wait_ge(s_W, rep * NSM + idx + 1)
                    a.dma_start(out=w_d[bh, ib * 128:(ib + 1) * 128, :],
                                in_=W_t[idx]).then_inc(s_wact, 16)


        @block.tensor
        def _(t):
            t.wait_ge(s_qk0b, 16)  # a2 rides in its own tiny chunk
            for rep in range(reps):
                for gg in range(GG):
                    bh, size, off = GLIST[gg]
                    if rep == 0 and bh == 1 and off == 0:
                        t.wait_ge(s_qka[1], 16)
                    if rep >= 1 and off == 0:
                        # sc bank reuse: previous rep's exp must have consumed it
                        t.wait_ge(s_E, (rep - 1) * NSM + NIB * (bh + 1))
                    t.wait_ge(s_S, rep * GG + gg + 1)
                    T = T_t[(rep * GG + gg) % 3]
                    for pl in range(size):
                        p = off + pl
                        for ib in range(NIB):
                            ins = nc.tensor.matmul(
                                sc_t[bh * NIB + ib][:, 2 * p:2 * p + 2],
                                T[:, pl * LQ + ib * 128: pl * LQ + (ib + 1) * 128],
                                a2(bh),
                                start=True, stop=True)
                    ins.then_inc(s_pe, 1)

    return nc


def _shard(q, k, a, mask):
    qf = q.reshape(B * H, LQ, D)
    kf = k.reshape(B * H, LK, D)
    mf = mask.reshape(B * H, LQ, LK)
    af = np.ascontiguousarray(
        np.broadcast_to(a.reshape(1, H, D), (B, H, D))).reshape(B * H, D)
    in_maps = []
    for c in range(NCORES):
        sl = slice(NBH * c, NBH * (c + 1))
        qT = qf[sl].transpose(0, 2, 1)                                # [NBH,64,LQ]
        kp = kf[sl].reshape(NBH, NPAIR, 2, D).transpose(0, 2, 3, 1)   # [NBH,2,D,NPAIR]
        qka = np.zeros((NBH, 128, QKA), np.float32)
        qka[:, 0:64, 0:LQ] = qT
        qka[:, 64:128, 0:LQ] = qT
        qka[:, :, LQ:LQ + NPAIR] = kp.reshape(NBH, 128, NPAIR)
        for j in range(NBH):
            qka[j, 0:64, LQ + NPAIR] = af[NBH * c + j]
            qka[j, 64:128, LQ + NPAIR + 1] = af[NBH * c + j]
        masku8 = np.ascontiguousarray(mf[sl]).astype(np.uint8)
        in_maps.append(dict(qka=qka, masku8=masku8))
    return in_maps


def kernel(q, k, attention, mask):
    global _built
    q = np.asarray(q, np.float32)
    k = np.asarray(k, np.float32)
    a = np.asarray(attention, np.float32)
    mask = np.asarray(mask).astype(bool)

    in_maps = _shard(q, k, a, mask)
    if _built is None:
        _built = _build()
    res = run_bass_kernel_spmd(_built, in_maps, core_ids=list(range(NCORES)))
    w = np.stack([res.results[c]["w"] for c in range(NCORES)], axis=0)
    return w.reshape(B, H, LQ, LK).astype(np.float32)



# revision 21
# speedup vs baseline: 1.0117x; 1.0117x over previous
"""GATv2 attention-weights kernel for 8 Trainium2 NeuronCores.

Problem (per full input):
    q: (2, 8, 384, 64) f32, k: (2, 8, 384, 64) f32,
    attention: (1, 8, 1, 1, 64) f32, mask: (2, 8, 384, 384) bool
    scores[b,h,i,j] = sum_d silu(q[b,h,i,d] + k[b,h,j,d]) * attention[h,d]
    out = softmax over j with mask (-inf before, 0 after)

Sharding: data-parallel over the 16 (b,h) pairs, 2 per core.

Per-core pipeline (raw bass, explicit semaphores; "jj,d" packing = two j
columns share the 128 partitions, d=64 each half):
  - ACT is the throughput floor (LQ*LK*D silu evals per (b,h) at 128
    lanes / 1.2 GHz, dtype-independent).  The first F pairs are computed
    directly on ACT via the fused activation bias operand
    silu(qT_rep + k_pair), removing the DVE round-trip from the ramp.
  - DVE builds the remaining T[(jj,d), i] = qT_rep + k_pair tiles in
    fp16 (4x DVE perf mode, 0.26 ns/elem) into a 192-pair fp16 ring;
    ACT computes silu in place per group.  Group sizes ramp so ACT
    never stalls, and are page-aligned to the ring (no wrap splits).
  - PE reduces over d with `a` folded into the weights:
    matmul(lhsT=T_pair fp16, rhs=a2 fp16) -> 2 score columns per pair,
    accumulated into per-(bh,i-block) PSUM banks in one long
    accumulation group (start on the first pair, stop on the last mask
    matmul).  The mask lands via extra fp16 matmuls (one per 64-column
    block: lhsT=maskT block, rhs=-1e4*I64), accumulating -1e4 into
    masked score positions (exp(-1e4) == 0), so there is no separate
    mask pass on any vector engine.
  - Softmax: a single activation-table switch to Exp after all silus
    (mid-stream Silu<->Exp switches are not supported by the backend).
    bh0's three tiles are one fused [128,3,384] exp; its row sums come
    from a DVE tensor_reduce that overlaps bh1's exps.  bh1's tiles use
    per-tile exps with accum_out so their sums are ready immediately.
    Reciprocal + scale on DVE; no per-row max (|scores| < 8).
  - Output DMAs alternate between the SP and ACT HWDGE queues in scale
    order (HWDGE setups and DMA transfers serialize device-wide, so the
    only lever is starting early and keeping the pipe busy).
"""

import numpy as np
from contextlib import ExitStack

import concourse.bass as bass
from concourse import mybir
from concourse.bass_utils import run_bass_kernel_spmd

B, H, LQ, LK, D = 2, 8, 384, 384, 64
NCORES = 8
NBH = (B * H) // NCORES        # 2 (b,h) pairs per core
NPAIR = LK // 2                # 192 j-pairs
NIB = LQ // 128                # 3 i-blocks
RING = 192                     # fp16 T ring capacity in pairs
F = 2                          # ACT-fused ramp pairs (bh0)

GROUPS0 = [3, 7, 16, 33, 67, 64]         # bh0 ramp (DVE 160 ns/pair)
GROUPS1 = [48, 48, 48, 36, 12]           # bh1 (page-aligned, short PE tail)
assert sum(GROUPS0) == NPAIR - F and sum(GROUPS1) == NPAIR
NG0, NG1 = len(GROUPS0), len(GROUPS1)
NPAIR_DVE = (NPAIR - F) + NPAIR          # DVE-built pairs per rep
NUNIT = None                             # set in _build (after segmentation)

_f32 = mybir.dt.float32
_f16 = mybir.dt.float16

_built = None  # cache across calls


def _build(reps=1):
    global NUNIT
    AF = mybir.ActivationFunctionType
    Alu = mybir.AluOpType

    nc = bass.Bass("TRN2", target_bir_lowering=False, debug=False,
                   num_devices=NCORES)

    qt_d = nc.dram_tensor("qt16", [NBH, 128, LQ + 2], _f16,
                          kind="ExternalInput").ap()
    kp_d = nc.dram_tensor("kp", [NBH, 128, NPAIR], _f32,
                          kind="ExternalInput").ap()
    mt_d = nc.dram_tensor("mt", [NBH, 128, NIB, LQ], _f16,
                          kind="ExternalInput").ap()
    mi_d = nc.dram_tensor("mi", [128, 64], _f16, kind="ExternalInput").ap()
    w_d = nc.dram_tensor("w", [NBH, LQ, LK], _f32, kind="ExternalOutput").ap()

    qt_t = [nc.alloc_sbuf_tensor(f"qt{bh}", [128, LQ + 2], _f16).ap()
            for bh in range(NBH)]
    kp_t = [nc.alloc_sbuf_tensor(f"kp{bh}", [128, NPAIR], _f32).ap()
            for bh in range(NBH)]
    mt_t = [nc.alloc_sbuf_tensor(f"mt{bh}", [128, NIB, LQ], _f16).ap()
            for bh in range(NBH)]
    mi_t = nc.alloc_sbuf_tensor("mi_t", [128, 64], _f16).ap()
    T_f = nc.alloc_sbuf_tensor("T_f", [128, F, LQ], _f16).ap()
    T_t = nc.alloc_sbuf_tensor("T_t", [128, RING, LQ], _f16).ap()
    E_t = [nc.alloc_sbuf_tensor(f"E{bh}", [128, NIB, LK], _f32).ap()
           for bh in range(NBH)]
    W_t = [nc.alloc_sbuf_tensor(f"W{bh}", [128, NIB, LK], _f32).ap()
           for bh in range(NBH)]
    sums_t = [nc.alloc_sbuf_tensor(f"sums{bh}", [128, NIB], _f32).ap()
              for bh in range(NBH)]
    r_t = [nc.alloc_sbuf_tensor(f"r{bh}", [128, NIB], _f32).ap()
           for bh in range(NBH)]
    sc_t = [nc.alloc_psum_tensor(f"sc{bh}", [128, NIB, 512], _f32).ap()
            for bh in range(NBH)]

    def qtrep(bh):
        return qt_t[bh][:, 0:LQ]

    def a2(bh):
        return qt_t[bh][:, LQ:LQ + 2]

    def kp(bh, p):
        return kp_t[bh][:, p:p + 1]

    # (bh, size, pair_offset, unit, ring_slot, reuse_wait_unit) per SEGMENT.
    # FIFO ring at pair granularity; groups crossing the ring end split.
    GL = []
    slot_owner = [None] * RING
    ro = 0

    def _alloc(bh, size, off):
        nonlocal ro
        while size > 0:
            seg = min(size, RING - ro)
            unit = len(GL) + F
            conflicts = [slot_owner[x] for x in range(ro, ro + seg)
                         if slot_owner[x] is not None]
            GL.append((bh, seg, off, unit, ro,
                       max(conflicts) if conflicts else None))
            for x in range(ro, ro + seg):
                slot_owner[x] = unit
            ro = (ro + seg) % RING
            off += seg
            size -= seg

    off = F
    for s in GROUPS0:
        _alloc(0, s, off)
        off += s
    if ro + GROUPS1[0] > RING:
        ro = 0          # skip waste slots so bh1 groups stay unsplit
    off = 0
    for s in GROUPS1:
        _alloc(1, s, off)
        off += s
    NUNIT = F + len(GL)

    with ExitStack() as ctx:
        s_q0 = ctx.enter_context(nc.semaphore("s_q0"))
        s_q1 = ctx.enter_context(nc.semaphore("s_q1"))
        s_q0b = ctx.enter_context(nc.semaphore("s_q0b"))
        s_mt = ctx.enter_context(nc.semaphore("s_mt"))
        s_T = ctx.enter_context(nc.semaphore("s_T"))
        s_S = ctx.enter_context(nc.semaphore("s_S"))
        s_pe = ctx.enter_context(nc.semaphore("s_pe"))
        s_pm = ctx.enter_context(nc.semaphore("s_pm"))
        s_E = ctx.enter_context(nc.semaphore("s_E"))
        s_W = ctx.enter_context(nc.semaphore("s_W"))
        s_wact = ctx.enter_context(nc.semaphore("s_wact"))
        s_wsp = ctx.enter_context(nc.semaphore("s_wsp"))
        block = ctx.enter_context(nc.Block())

        # scale emission order (drives s_W numbering): t3=1, t4=2, t0=3,
        # t1=4, t2=5, t5=6.  (bh, ib, s_W) per DMA, split across queues:
        SP_DMAS = ((1, 0, 1), (0, 0, 3), (0, 2, 5))     # t3, t0, t2
        ACT_DMAS = ((1, 1, 2), (0, 1, 4), (1, 2, 6))    # t4, t1, t5

        @block.sync
        def _(sp):
            sp.dma_start(out=qt_t[0], in_=qt_d[0]).then_inc(s_q0, 16)
            sp.dma_start(out=kp_t[0][:, 0:8],
                         in_=kp_d[0][:, 0:8]).then_inc(s_q0, 16)
            sp.dma_start(out=kp_t[0][:, 8:NPAIR],
                         in_=kp_d[0][:, 8:NPAIR]).then_inc(s_q0b, 16)
            sp.dma_start(out=qt_t[1], in_=qt_d[1]).then_inc(s_q1, 16)
            sp.dma_start(out=kp_t[1], in_=kp_d[1]).then_inc(s_q1, 16)
            sp.dma_start(out=mi_t, in_=mi_d).then_inc(s_mt, 16)
            for bh in range(NBH):
                sp.dma_start(out=mt_t[bh], in_=mt_d[bh]).then_inc(s_mt, 16)
            for rep in range(reps):
                for bh, ib, sw in SP_DMAS:
                    sp.wait_ge(s_W, 6 * rep + sw)
                    sp.dma_start(out=w_d[bh][ib * 128:(ib + 1) * 128, :],
                                 in_=W_t[bh][:, ib, :]).then_inc(s_wsp, 16)
            sp.wait_ge(s_wsp, 16 * len(SP_DMAS) * reps)
            sp.wait_ge(s_wact, 16 * len(ACT_DMAS) * reps)

        @block.scalar
        def _(a):
            a.wait_ge(s_q0, 32)
            for rep in range(reps):
                for p in range(F):
                    if rep >= 1:
                        a.wait_ge(s_pe, (rep - 1) * NUNIT + p + 1)
                    a.activation(T_f[:, p, :], qtrep(0), AF.Silu,
                                 bias=kp(0, p)).then_inc(s_S, 1)
                for bh, size, off, unit, ro_, _rw in GL:
                    a.wait_ge(s_T, rep * NPAIR_DVE +
                              (off + size - F if bh == 0
                               else (NPAIR - F) + off + size))
                    a.activation(T_t[:, ro_:ro_ + size, :],
                                 T_t[:, ro_:ro_ + size, :],
                                 AF.Silu).then_inc(s_S, 1)
                # ---- softmax (one table switch per rep) ----
                a.wait_ge(s_pm, 12 * rep + 6)
                if rep >= 1:
                    a.wait_ge(s_W, 6 * (rep - 1) + 6)   # E0 reuse
                a.activation(E_t[0], sc_t[0][:, :, 0:LK],
                             AF.Exp).then_inc(s_E, 1)
                a.wait_ge(s_pm, 12 * rep + 12)
                for t in range(NIB):
                    a.activation(E_t[1][:, t, :], sc_t[1][:, t, 0:LK],
                                 AF.Exp,
                                 accum_out=sums_t[1][:, t:t + 1]
                                 ).then_inc(s_E, 1)
                # output DMAs on ACT's queue (SEQ free after the exps)
                for bh, ib, sw in ACT_DMAS:
                    a.wait_ge(s_W, 6 * rep + sw)
                    a.dma_start(out=w_d[bh][ib * 128:(ib + 1) * 128, :],
                                in_=W_t[bh][:, ib, :]).then_inc(s_wact, 16)

        @block.vector
        def _(v):
            v.wait_ge(s_q0, 32)
            for rep in range(reps):
                for gi, (bh, size, off, unit, ro_, rw) in enumerate(GL):
                    if bh == 1 and off == 0 and rep == 0:
                        v.wait_ge(s_q1, 32)
                    if rw is not None:
                        v.wait_ge(s_pe, rep * NUNIT + rw + 1)
                    elif rep >= 1:
                        v.wait_ge(s_pe, (rep - 1) * NUNIT + unit + 1)
                    for pl in range(size):
                        p = off + pl
                        if bh == 0 and rep == 0 and p == 8:
                            v.wait_ge(s_q0b, 16)
                        v.tensor_scalar_add(T_t[:, ro_ + pl, :], qtrep(bh),
                                            kp(bh, p)).then_inc(s_T, 1)
                # ---- softmax finalize ----
                v.wait_ge(s_E, 4 * rep + 1)
                if rep >= 1:
                    v.wait_ge(s_wact, 16 * len(ACT_DMAS) * rep)
                    v.wait_ge(s_wsp, 16 * len(SP_DMAS) * rep)
                v.tensor_reduce(sums_t[0], E_t[0], mybir.AxisListType.X,
                                Alu.add)
                v.wait_ge(s_E, 4 * rep + 3)           # acc3, acc4 done
                v.reciprocal(r_t[1][:, 0:2], sums_t[1][:, 0:2])
                v.reciprocal(r_t[0], sums_t[0])
                v.drain()
                # s_W order: t3, t4, t0, t1, t2, t5
                v.tensor_scalar_mul(W_t[1][:, 0, :], E_t[1][:, 0, :],
                                    r_t[1][:, 0:1]).then_inc(s_W, 1)
                v.tensor_scalar_mul(W_t[1][:, 1, :], E_t[1][:, 1, :],
                                    r_t[1][:, 1:2]).then_inc(s_W, 1)
                for t in range(NIB):
                    v.tensor_scalar_mul(W_t[0][:, t, :], E_t[0][:, t, :],
                                        r_t[0][:, t:t + 1]).then_inc(s_W, 1)
                v.wait_ge(s_E, 4 * rep + 4)           # acc5 done
                v.reciprocal(r_t[1][:, 2:3], sums_t[1][:, 2:3])
                v.drain()
                v.tensor_scalar_mul(W_t[1][:, 2, :], E_t[1][:, 2, :],
                                    r_t[1][:, 2:3]).then_inc(s_W, 1)

        @block.tensor
        def _(t):
            t.wait_ge(s_q0, 32)
            t.wait_ge(s_mt, 48)
            for rep in range(reps):
                for bh in range(NBH):
                    if rep >= 1:
                        t.wait_ge(s_E, 4 * (rep - 1) + (1 if bh == 0 else 4))
                    if bh == 1 and rep == 0:
                        t.wait_ge(s_q1, 32)
                    units = [(u, s, o, r) for (b, s, o, u, r, _w) in GL
                             if b == bh]
                    if bh == 0:
                        units = [(u, 1, u, None) for u in range(F)] + units
                    for unit, size, off, ro_ in units:
                        t.wait_ge(s_S, rep * NUNIT + unit + 1)
                        ins = None
                        for pl in range(size):
                            p = off + pl
                            lhsT = (T_f[:, p, :] if unit < F
                                    else T_t[:, ro_ + pl, :])
                            for ib in range(NIB):
                                ins = nc.tensor.matmul(
                                    sc_t[bh][:, ib, 2 * p:2 * p + 2],
                                    lhsT[:, ib * 128:(ib + 1) * 128],
                                    a2(bh), start=(p == 0), stop=False)
                            if (2 * p + 2) % 64 == 0:
                                J = (2 * p + 2) // 64 - 1
                                base, jb = (64 * J) % 128, (64 * J) // 128
                                for ib in range(NIB):
                                    mm = nc.tensor.matmul(
                                        sc_t[bh][:, ib, 64 * J:64 * J + 64],
                                        mt_t[bh][base:base + 64, jb,
                                                 ib * 128:(ib + 1) * 128],
                                        mi_t[base:base + 64, :],
                                        start=False, stop=(J == 5))
                                mm.then_inc(s_pm, 1)
                        ins.then_inc(s_pe, 1)

    return nc


def _shard(q, k, a, mask):
    qf = q.reshape(B * H, LQ, D)
    kf = k.reshape(B * H, LK, D)
    mf = mask.reshape(B * H, LQ, LK)
    af = np.ascontiguousarray(
        np.broadcast_to(a.reshape(1, H, D), (B, H, D))).reshape(B * H, D)
    mi = np.zeros((128, 64), np.float32)
    mi[0:64, :] = np.eye(64) * -1e4
    mi[64:128, :] = np.eye(64) * -1e4
    mi = mi.astype(np.float16)
    in_maps = []
    for c in range(NCORES):
        sl = slice(NBH * c, NBH * (c + 1))
        qT = qf[sl].transpose(0, 2, 1)                              # [NBH,64,LQ]
        qt16 = np.zeros((NBH, 128, LQ + 2), np.float32)
        qt16[:, 0:64, 0:LQ] = qT
        qt16[:, 64:128, 0:LQ] = qT
        for j in range(NBH):
            qt16[j, 0:64, LQ] = af[NBH * c + j]
            qt16[j, 64:128, LQ + 1] = af[NBH * c + j]
        kp_ = kf[sl].reshape(NBH, NPAIR, 2, D).transpose(0, 2, 3, 1)
        kp_ = np.ascontiguousarray(kp_.reshape(NBH, 128, NPAIR),
                                   dtype=np.float32)
        # maskT[bh][j%128, j//128, i] = mask[bh][i, j]
        m = mf[sl].astype(np.float32)                               # [NBH,i,j]
        mt = m.transpose(0, 2, 1).reshape(NBH, NIB, 128, LQ)
        mt = np.ascontiguousarray(mt.transpose(0, 2, 1, 3))
        in_maps.append(dict(qt16=qt16.astype(np.float16), kp=kp_,
                            mt=mt.astype(np.float16), mi=mi))
    return in_maps


def kernel(q, k, attention, mask):
    global _built
    q = np.asarray(q, np.float32)
    k = np.asarray(k, np.float32)
    a = np.asarray(attention, np.float32)
    mask = np.asarray(mask).astype(bool)

    in_maps = _shard(q, k, a, mask)
    if _built is None:
        _built = _build()
    res = run_bass_kernel_spmd(_built, in_maps, core_ids=list(range(NCORES)))
    w = np.stack([res.results[c]["w"] for c in range(NCORES)], axis=0)
    return w.reshape(B, H, LQ, LK).astype(np.float32)


# revision 22
# speedup vs baseline: 1.0126x; 1.0009x over previous
"""GATv2 attention-weights kernel for 8 Trainium2 NeuronCores.

Problem (per full input):
    q: (2, 8, 384, 64) f32, k: (2, 8, 384, 64) f32,
    attention: (1, 8, 1, 1, 64) f32, mask: (2, 8, 384, 384) bool
    scores[b,h,i,j] = sum_d silu(q[b,h,i,d] + k[b,h,j,d]) * attention[h,d]
    out = softmax over j with mask (-inf before, 0 after)

Sharding: data-parallel over the 16 (b,h) pairs, 2 per core.

Per-core pipeline (raw bass, explicit semaphores; "jj,d" packing = two j
columns share the 128 partitions, d=64 each half):
  - ACT is the throughput floor (LQ*LK*D silu evals per (b,h) at 128
    lanes / 1.2 GHz, dtype-independent).  The first F pairs are computed
    directly on ACT via the fused activation bias operand
    silu(qT_rep + k_pair), removing the DVE round-trip from the ramp.
  - DVE builds the remaining T[(jj,d), i] = qT_rep + k_pair tiles in
    fp16 (4x DVE perf mode, 0.26 ns/elem) into a 192-pair fp16 ring;
    ACT computes silu in place per group.  Group sizes ramp so ACT
    never stalls, and are page-aligned to the ring (no wrap splits).
  - PE reduces over d with `a` folded into the weights:
    matmul(lhsT=T_pair fp16, rhs=a2 fp16) -> 2 score columns per pair,
    accumulated into per-(bh,i-block) PSUM banks in one long
    accumulation group (start on the first pair, stop on the last mask
    matmul).  The mask lands via extra fp16 matmuls (one per 64-column
    block: lhsT=maskT block, rhs=-1e4*I64), accumulating -1e4 into
    masked score positions (exp(-1e4) == 0), so there is no separate
    mask pass on any vector engine.
  - Softmax: a single activation-table switch to Exp after all silus
    (mid-stream Silu<->Exp switches are not supported by the backend).
    bh0's three tiles are one fused [128,3,384] exp; its row sums come
    from a DVE tensor_reduce that overlaps bh1's exps.  bh1's tiles use
    per-tile exps with accum_out so their sums are ready immediately.
    Reciprocal + scale on DVE; no per-row max (|scores| < 8).
  - Output DMAs alternate between the SP and ACT HWDGE queues in scale
    order (HWDGE setups and DMA transfers serialize device-wide, so the
    only lever is starting early and keeping the pipe busy).
"""

import numpy as np
from contextlib import ExitStack

import concourse.bass as bass
from concourse import mybir
from concourse.bass_utils import run_bass_kernel_spmd

B, H, LQ, LK, D = 2, 8, 384, 384, 64
NCORES = 8
NBH = (B * H) // NCORES        # 2 (b,h) pairs per core
NPAIR = LK // 2                # 192 j-pairs
NIB = LQ // 128                # 3 i-blocks
RING = 192                     # fp16 T ring capacity in pairs
F = 2                          # ACT-fused ramp pairs (bh0)

GROUPS0 = [3, 7, 16, 33, 67, 64]         # bh0 ramp (DVE 160 ns/pair)
GROUPS1 = [48, 48, 48, 36, 12]           # bh1 (page-aligned, short PE tail)
assert sum(GROUPS0) == NPAIR - F and sum(GROUPS1) == NPAIR
NG0, NG1 = len(GROUPS0), len(GROUPS1)
NPAIR_DVE = (NPAIR - F) + NPAIR          # DVE-built pairs per rep
NUNIT = None                             # set in _build (after segmentation)

_f32 = mybir.dt.float32
_f16 = mybir.dt.float16

_built = None  # cache across calls


def _build(reps=1):
    global NUNIT
    AF = mybir.ActivationFunctionType
    Alu = mybir.AluOpType

    nc = bass.Bass("TRN2", target_bir_lowering=False, debug=False,
                   num_devices=NCORES)

    qt_d = nc.dram_tensor("qt16", [NBH, 128, LQ + 2], _f16,
                          kind="ExternalInput").ap()
    kp_d = nc.dram_tensor("kp", [NBH, 128, NPAIR], _f32,
                          kind="ExternalInput").ap()
    mt_d = nc.dram_tensor("mt", [NBH, 128, NIB, LQ], _f16,
                          kind="ExternalInput").ap()
    mi_d = nc.dram_tensor("mi", [128, 64], _f16, kind="ExternalInput").ap()
    # w in "transposed" layout [part, ib, j]; the host gather undoes it
    w_d = nc.dram_tensor("w", [NBH, 128, NIB, LK], _f32,
                         kind="ExternalOutput").ap()

    qt_t = [nc.alloc_sbuf_tensor(f"qt{bh}", [128, LQ + 2], _f16).ap()
            for bh in range(NBH)]
    kp_t = [nc.alloc_sbuf_tensor(f"kp{bh}", [128, NPAIR], _f32).ap()
            for bh in range(NBH)]
    mt_t = [nc.alloc_sbuf_tensor(f"mt{bh}", [128, NIB, LQ], _f16).ap()
            for bh in range(NBH)]
    mi_t = nc.alloc_sbuf_tensor("mi_t", [128, 64], _f16).ap()
    T_f = nc.alloc_sbuf_tensor("T_f", [128, F, LQ], _f16).ap()
    T_t = nc.alloc_sbuf_tensor("T_t", [128, RING, LQ], _f16).ap()
    E_t = [nc.alloc_sbuf_tensor(f"E{bh}", [128, NIB, LK], _f32).ap()
           for bh in range(NBH)]
    W_t = [nc.alloc_sbuf_tensor(f"W{bh}", [128, NIB, LK], _f32).ap()
           for bh in range(NBH)]
    sums_t = [nc.alloc_sbuf_tensor(f"sums{bh}", [128, NIB], _f32).ap()
              for bh in range(NBH)]
    r_t = [nc.alloc_sbuf_tensor(f"r{bh}", [128, NIB], _f32).ap()
           for bh in range(NBH)]
    sc_t = [nc.alloc_psum_tensor(f"sc{bh}", [128, NIB, 512], _f32).ap()
            for bh in range(NBH)]

    def qtrep(bh):
        return qt_t[bh][:, 0:LQ]

    def a2(bh):
        return qt_t[bh][:, LQ:LQ + 2]

    def kp(bh, p):
        return kp_t[bh][:, p:p + 1]

    # (bh, size, pair_offset, unit, ring_slot, reuse_wait_unit) per SEGMENT.
    # FIFO ring at pair granularity; groups crossing the ring end split.
    GL = []
    slot_owner = [None] * RING
    ro = 0

    def _alloc(bh, size, off):
        nonlocal ro
        while size > 0:
            seg = min(size, RING - ro)
            unit = len(GL) + F
            conflicts = [slot_owner[x] for x in range(ro, ro + seg)
                         if slot_owner[x] is not None]
            GL.append((bh, seg, off, unit, ro,
                       max(conflicts) if conflicts else None))
            for x in range(ro, ro + seg):
                slot_owner[x] = unit
            ro = (ro + seg) % RING
            off += seg
            size -= seg

    off = F
    for s in GROUPS0:
        _alloc(0, s, off)
        off += s
    if ro + GROUPS1[0] > RING:
        ro = 0          # skip waste slots so bh1 groups stay unsplit
    off = 0
    for s in GROUPS1:
        _alloc(1, s, off)
        off += s
    NUNIT = F + len(GL)

    with ExitStack() as ctx:
        s_q0 = ctx.enter_context(nc.semaphore("s_q0"))
        s_q1 = ctx.enter_context(nc.semaphore("s_q1"))
        s_q0b = ctx.enter_context(nc.semaphore("s_q0b"))
        s_mt = ctx.enter_context(nc.semaphore("s_mt"))
        s_T = ctx.enter_context(nc.semaphore("s_T"))
        s_S = ctx.enter_context(nc.semaphore("s_S"))
        s_pe = ctx.enter_context(nc.semaphore("s_pe"))
        s_pm = ctx.enter_context(nc.semaphore("s_pm"))
        s_E = ctx.enter_context(nc.semaphore("s_E"))
        s_W = ctx.enter_context(nc.semaphore("s_W"))
        s_wact = ctx.enter_context(nc.semaphore("s_wact"))
        s_wsp = ctx.enter_context(nc.semaphore("s_wsp"))
        block = ctx.enter_context(nc.Block())

        # scale emission order (drives s_W numbering): t3=1, t4=2, t0=3,
        # t1=4, t2=5, t5=6.  SP carries bh1 per-tile; ACT carries bh0 as
        # one merged DMA (w layout is [part, ib, j] so shapes line up).

        @block.sync
        def _(sp):
            sp.dma_start(out=qt_t[0], in_=qt_d[0]).then_inc(s_q0, 16)
            sp.dma_start(out=kp_t[0][:, 0:8],
                         in_=kp_d[0][:, 0:8]).then_inc(s_q0, 16)
            sp.dma_start(out=kp_t[0][:, 8:NPAIR],
                         in_=kp_d[0][:, 8:NPAIR]).then_inc(s_q0b, 16)
            sp.dma_start(out=qt_t[1], in_=qt_d[1]).then_inc(s_q1, 16)
            sp.dma_start(out=kp_t[1], in_=kp_d[1]).then_inc(s_q1, 16)
            sp.dma_start(out=mi_t, in_=mi_d).then_inc(s_mt, 16)
            for bh in range(NBH):
                sp.dma_start(out=mt_t[bh], in_=mt_d[bh]).then_inc(s_mt, 16)
            for rep in range(reps):
                for ib, sw in ((0, 1), (1, 2), (2, 6)):   # t3, t4, t5
                    sp.wait_ge(s_W, 6 * rep + sw)
                    sp.dma_start(out=w_d[1][:, ib, :],
                                 in_=W_t[1][:, ib, :]).then_inc(s_wsp, 16)
            sp.wait_ge(s_wsp, 48 * reps)
            sp.wait_ge(s_wact, 16 * reps)

        @block.scalar
        def _(a):
            a.wait_ge(s_q0, 32)
            for rep in range(reps):
                for p in range(F):
                    if rep >= 1:
                        a.wait_ge(s_pe, (rep - 1) * NUNIT + p + 1)
                    a.activation(T_f[:, p, :], qtrep(0), AF.Silu,
                                 bias=kp(0, p)).then_inc(s_S, 1)
                for bh, size, off, unit, ro_, _rw in GL:
                    a.wait_ge(s_T, rep * NPAIR_DVE +
                              (off + size - F if bh == 0
                               else (NPAIR - F) + off + size))
                    a.activation(T_t[:, ro_:ro_ + size, :],
                                 T_t[:, ro_:ro_ + size, :],
                                 AF.Silu).then_inc(s_S, 1)
                # ---- softmax (one table switch per rep) ----
                a.wait_ge(s_pm, 12 * rep + 6)
                if rep >= 1:
                    a.wait_ge(s_W, 6 * (rep - 1) + 6)   # E0 reuse
                a.activation(E_t[0], sc_t[0][:, :, 0:LK],
                             AF.Exp).then_inc(s_E, 1)
                a.wait_ge(s_pm, 12 * rep + 12)
                for t in range(NIB):
                    a.activation(E_t[1][:, t, :], sc_t[1][:, t, 0:LK],
                                 AF.Exp,
                                 accum_out=sums_t[1][:, t:t + 1]
                                 ).then_inc(s_E, 1)
                # bh0's three tiles as ONE output DMA on ACT's queue
                a.wait_ge(s_W, 6 * rep + 5)
                a.dma_start(out=w_d[0], in_=W_t[0]).then_inc(s_wact, 16)

        @block.vector
        def _(v):
            v.wait_ge(s_q0, 32)
            for rep in range(reps):
                for gi, (bh, size, off, unit, ro_, rw) in enumerate(GL):
                    if bh == 1 and off == 0 and rep == 0:
                        v.wait_ge(s_q1, 32)
                    if rw is not None:
                        v.wait_ge(s_pe, rep * NUNIT + rw + 1)
                    elif rep >= 1:
                        v.wait_ge(s_pe, (rep - 1) * NUNIT + unit + 1)
                    for pl in range(size):
                        p = off + pl
                        if bh == 0 and rep == 0 and p == 8:
                            v.wait_ge(s_q0b, 16)
                        v.tensor_scalar_add(T_t[:, ro_ + pl, :], qtrep(bh),
                                            kp(bh, p)).then_inc(s_T, 1)
                # ---- softmax finalize ----
                v.wait_ge(s_E, 4 * rep + 1)
                if rep >= 1:
                    v.wait_ge(s_wact, 16 * rep)
                    v.wait_ge(s_wsp, 48 * rep)
                v.tensor_reduce(sums_t[0], E_t[0], mybir.AxisListType.X,
                                Alu.add)
                v.wait_ge(s_E, 4 * rep + 3)           # acc3, acc4 done
                v.reciprocal(r_t[1][:, 0:2], sums_t[1][:, 0:2])
                v.reciprocal(r_t[0], sums_t[0])
                v.drain()
                # s_W order: t3, t4, t0, t1, t2, t5
                v.tensor_scalar_mul(W_t[1][:, 0, :], E_t[1][:, 0, :],
                                    r_t[1][:, 0:1]).then_inc(s_W, 1)
                v.tensor_scalar_mul(W_t[1][:, 1, :], E_t[1][:, 1, :],
                                    r_t[1][:, 1:2]).then_inc(s_W, 1)
                for t in range(NIB):
                    v.tensor_scalar_mul(W_t[0][:, t, :], E_t[0][:, t, :],
                                        r_t[0][:, t:t + 1]).then_inc(s_W, 1)
                v.wait_ge(s_E, 4 * rep + 4)           # acc5 done
                v.reciprocal(r_t[1][:, 2:3], sums_t[1][:, 2:3])
                v.drain()
                v.tensor_scalar_mul(W_t[1][:, 2, :], E_t[1][:, 2, :],
                                    r_t[1][:, 2:3]).then_inc(s_W, 1)

        @block.tensor
        def _(t):
            t.wait_ge(s_q0, 32)
            t.wait_ge(s_mt, 48)
            for rep in range(reps):
                for bh in range(NBH):
                    if rep >= 1:
                        t.wait_ge(s_E, 4 * (rep - 1) + (1 if bh == 0 else 4))
                    if bh == 1 and rep == 0:
                        t.wait_ge(s_q1, 32)
                    units = [(u, s, o, r) for (b, s, o, u, r, _w) in GL
                             if b == bh]
                    if bh == 0:
                        units = [(u, 1, u, None) for u in range(F)] + units
                    for unit, size, off, ro_ in units:
                        t.wait_ge(s_S, rep * NUNIT + unit + 1)
                        ins = None
                        for pl in range(size):
                            p = off + pl
                            lhsT = (T_f[:, p, :] if unit < F
                                    else T_t[:, ro_ + pl, :])
                            for ib in range(NIB):
                                ins = nc.tensor.matmul(
                                    sc_t[bh][:, ib, 2 * p:2 * p + 2],
                                    lhsT[:, ib * 128:(ib + 1) * 128],
                                    a2(bh), start=(p == 0), stop=False)
                            if (2 * p + 2) % 64 == 0:
                                J = (2 * p + 2) // 64 - 1
                                base, jb = (64 * J) % 128, (64 * J) // 128
                                for ib in range(NIB):
                                    mm = nc.tensor.matmul(
                                        sc_t[bh][:, ib, 64 * J:64 * J + 64],
                                        mt_t[bh][base:base + 64, jb,
                                                 ib * 128:(ib + 1) * 128],
                                        mi_t[base:base + 64, :],
                                        start=False, stop=(J == 5))
                                mm.then_inc(s_pm, 1)
                        ins.then_inc(s_pe, 1)

    return nc


def _shard(q, k, a, mask):
    qf = q.reshape(B * H, LQ, D)
    kf = k.reshape(B * H, LK, D)
    mf = mask.reshape(B * H, LQ, LK)
    af = np.ascontiguousarray(
        np.broadcast_to(a.reshape(1, H, D), (B, H, D))).reshape(B * H, D)
    mi = np.zeros((128, 64), np.float32)
    mi[0:64, :] = np.eye(64) * -1e4
    mi[64:128, :] = np.eye(64) * -1e4
    mi = mi.astype(np.float16)
    in_maps = []
    for c in range(NCORES):
        sl = slice(NBH * c, NBH * (c + 1))
        qT = qf[sl].transpose(0, 2, 1)                              # [NBH,64,LQ]
        qt16 = np.zeros((NBH, 128, LQ + 2), np.float32)
        qt16[:, 0:64, 0:LQ] = qT
        qt16[:, 64:128, 0:LQ] = qT
        for j in range(NBH):
            qt16[j, 0:64, LQ] = af[NBH * c + j]
            qt16[j, 64:128, LQ + 1] = af[NBH * c + j]
        kp_ = kf[sl].reshape(NBH, NPAIR, 2, D).transpose(0, 2, 3, 1)
        kp_ = np.ascontiguousarray(kp_.reshape(NBH, 128, NPAIR),
                                   dtype=np.float32)
        # maskT[bh][j%128, j//128, i] = mask[bh][i, j]
        m = mf[sl].astype(np.float32)                               # [NBH,i,j]
        mt = m.transpose(0, 2, 1).reshape(NBH, NIB, 128, LQ)
        mt = np.ascontiguousarray(mt.transpose(0, 2, 1, 3))
        in_maps.append(dict(qt16=qt16.astype(np.float16), kp=kp_,
                            mt=mt.astype(np.float16), mi=mi))
    return in_maps


def kernel(q, k, attention, mask):
    global _built
    q = np.asarray(q, np.float32)
    k = np.asarray(k, np.float32)
    a = np.asarray(attention, np.float32)
    mask = np.asarray(mask).astype(bool)

    in_maps = _shard(q, k, a, mask)
    if _built is None:
        _built = _build()
    res = run_bass_kernel_spmd(_built, in_maps, core_ids=list(range(NCORES)))
    w = np.stack([res.results[c]["w"] for c in range(NCORES)], axis=0)
    # undo the [part, ib, j] device layout -> [i, j]
    w = w.transpose(0, 1, 3, 2, 4).reshape(NCORES, NBH, LQ, LK)
    return w.reshape(B, H, LQ, LK).astype(np.float32)


# revision 25
# speedup vs baseline: 1.0136x; 1.0009x over previous
"""GATv2 attention-weights kernel for 8 Trainium2 NeuronCores.

Problem (per full input):
    q: (2, 8, 384, 64) f32, k: (2, 8, 384, 64) f32,
    attention: (1, 8, 1, 1, 64) f32, mask: (2, 8, 384, 384) bool
    scores[b,h,i,j] = sum_d silu(q[b,h,i,d] + k[b,h,j,d]) * attention[h,d]
    out = softmax over j with mask (-inf before, 0 after)

Sharding: data-parallel over the 16 (b,h) pairs, 2 per core.

Per-core pipeline (raw bass, explicit semaphores; "jj,d" packing = two j
columns share the 128 partitions, d=64 each half):
  - ACT is the throughput floor (LQ*LK*D silu evals per (b,h) at 128
    lanes / 1.2 GHz, dtype-independent).  The first F pairs are computed
    directly on ACT via the fused activation bias operand
    silu(qT_rep + k_pair), removing the DVE round-trip from the ramp.
  - DVE builds the remaining T[(jj,d), i] = qT_rep + k_pair tiles in
    fp16 (4x DVE perf mode, 0.26 ns/elem) into a 192-pair fp16 ring;
    ACT computes silu in place per group.  Group sizes ramp so ACT
    never stalls, and are page-aligned to the ring (no wrap splits).
  - PE reduces over d with `a` folded into the weights:
    matmul(lhsT=T_pair fp16, rhs=a2 fp16) -> 2 score columns per pair,
    accumulated into per-(bh,i-block) PSUM banks in one long
    accumulation group (start on the first pair, stop on the last mask
    matmul).  The mask lands via extra fp16 matmuls (one per 64-column
    block: lhsT=maskT block, rhs=-1e4*I64), accumulating -1e4 into
    masked score positions (exp(-1e4) == 0), so there is no separate
    mask pass on any vector engine.
  - Softmax: a single activation-table switch to Exp after all silus
    (mid-stream Silu<->Exp switches are not supported by the backend).
    bh0's three tiles are one fused [128,3,384] exp; its row sums come
    from a DVE tensor_reduce that overlaps bh1's exps.  bh1's tiles use
    per-tile exps with accum_out so their sums are ready immediately.
    Reciprocal + scale on DVE; no per-row max (|scores| < 8).
  - Output DMAs alternate between the SP and ACT HWDGE queues in scale
    order (HWDGE setups and DMA transfers serialize device-wide, so the
    only lever is starting early and keeping the pipe busy).
"""

import numpy as np
from contextlib import ExitStack

import concourse.bass as bass
from concourse import mybir
from concourse.bass_utils import run_bass_kernel_spmd

B, H, LQ, LK, D = 2, 8, 384, 384, 64
NCORES = 8
NBH = (B * H) // NCORES        # 2 (b,h) pairs per core
NPAIR = LK // 2                # 192 j-pairs
NIB = LQ // 128                # 3 i-blocks
RING = 192                     # fp16 T ring capacity in pairs
F = 2                          # ACT-fused ramp pairs (bh0)

GROUPS0 = [3, 7, 16, 33, 67, 64]         # bh0 ramp (DVE 160 ns/pair)
GROUPS1 = [48, 48, 48, 36, 12]           # bh1 (page-aligned, short PE tail)
assert sum(GROUPS0) == NPAIR - F and sum(GROUPS1) == NPAIR
NG0, NG1 = len(GROUPS0), len(GROUPS1)
NPAIR_DVE = (NPAIR - F) + NPAIR          # DVE-built pairs per rep
NUNIT = None                             # set in _build (after segmentation)

_f32 = mybir.dt.float32
_f16 = mybir.dt.float16

_built = None  # cache across calls


def _build(reps=1):
    global NUNIT
    AF = mybir.ActivationFunctionType
    Alu = mybir.AluOpType

    nc = bass.Bass("TRN2", target_bir_lowering=False, debug=False,
                   num_devices=NCORES)

    qt_d = nc.dram_tensor("qt16", [NBH, 128, LQ + 2], _f16,
                          kind="ExternalInput").ap()
    kp_d = nc.dram_tensor("kp", [NBH, 128, NPAIR], _f32,
                          kind="ExternalInput").ap()
    mt_d = nc.dram_tensor("mt", [NBH, 128, NIB, LQ], _f16,
                          kind="ExternalInput").ap()
    mi_d = nc.dram_tensor("mi", [128, 64], _f16, kind="ExternalInput").ap()
    # w in "transposed" layout [part, ib, j]; the host gather undoes it
    w_d = nc.dram_tensor("w", [NBH, 128, NIB, LK], _f32,
                         kind="ExternalOutput").ap()

    qt_t = [nc.alloc_sbuf_tensor(f"qt{bh}", [128, LQ + 2], _f16).ap()
            for bh in range(NBH)]
    kp_t = [nc.alloc_sbuf_tensor(f"kp{bh}", [128, NPAIR], _f32).ap()
            for bh in range(NBH)]
    mt_t = [nc.alloc_sbuf_tensor(f"mt{bh}", [128, NIB, LQ], _f16).ap()
            for bh in range(NBH)]
    mi_t = nc.alloc_sbuf_tensor("mi_t", [128, 64], _f16).ap()
    T_f = nc.alloc_sbuf_tensor("T_f", [128, F, LQ], _f16).ap()
    T_t = nc.alloc_sbuf_tensor("T_t", [128, RING, LQ], _f16).ap()
    E_t = [nc.alloc_sbuf_tensor(f"E{bh}", [128, NIB, LK], _f32).ap()
           for bh in range(NBH)]
    W_t = [nc.alloc_sbuf_tensor(f"W{bh}", [128, NIB, LK], _f32).ap()
           for bh in range(NBH)]
    sums_t = [nc.alloc_sbuf_tensor(f"sums{bh}", [128, NIB], _f32).ap()
              for bh in range(NBH)]
    r_t = [nc.alloc_sbuf_tensor(f"r{bh}", [128, NIB], _f32).ap()
           for bh in range(NBH)]
    sc_t = [nc.alloc_psum_tensor(f"sc{bh}", [128, NIB, 512], _f32).ap()
            for bh in range(NBH)]

    def qtrep(bh):
        return qt_t[bh][:, 0:LQ]

    def a2(bh):
        return qt_t[bh][:, LQ:LQ + 2]

    def kp(bh, p):
        return kp_t[bh][:, p:p + 1]

    # (bh, size, pair_offset, unit, ring_slot, reuse_wait_unit) per SEGMENT.
    # FIFO ring at pair granularity; groups crossing the ring end split.
    GL = []
    slot_owner = [None] * RING
    ro = 0

    def _alloc(bh, size, off):
        nonlocal ro
        while size > 0:
            seg = min(size, RING - ro)
            unit = len(GL) + F
            conflicts = [slot_owner[x] for x in range(ro, ro + seg)
                         if slot_owner[x] is not None]
            GL.append((bh, seg, off, unit, ro,
                       max(conflicts) if conflicts else None))
            for x in range(ro, ro + seg):
                slot_owner[x] = unit
            ro = (ro + seg) % RING
            off += seg
            size -= seg

    off = F
    for s in GROUPS0:
        _alloc(0, s, off)
        off += s
    if ro + GROUPS1[0] > RING:
        ro = 0          # skip waste slots so bh1 groups stay unsplit
    off = 0
    for s in GROUPS1:
        _alloc(1, s, off)
        off += s
    NUNIT = F + len(GL)

    with ExitStack() as ctx:
        s_q0 = ctx.enter_context(nc.semaphore("s_q0"))
        s_q1 = ctx.enter_context(nc.semaphore("s_q1"))
        s_q0b = ctx.enter_context(nc.semaphore("s_q0b"))
        s_mt = ctx.enter_context(nc.semaphore("s_mt"))
        s_T = ctx.enter_context(nc.semaphore("s_T"))
        s_S = ctx.enter_context(nc.semaphore("s_S"))
        s_pe = ctx.enter_context(nc.semaphore("s_pe"))
        s_pm = ctx.enter_context(nc.semaphore("s_pm"))
        s_E = ctx.enter_context(nc.semaphore("s_E"))
        s_W = ctx.enter_context(nc.semaphore("s_W"))
        s_wact = ctx.enter_context(nc.semaphore("s_wact"))
        s_wsp = ctx.enter_context(nc.semaphore("s_wsp"))
        block = ctx.enter_context(nc.Block())

        # scale emission order (drives s_W numbering): t3=1, t4=2, t0=3,
        # t1=4, t2=5, t5=6.  SP carries bh1 per-tile; ACT carries bh0 as
        # one merged DMA (w layout is [part, ib, j] so shapes line up).

        @block.sync
        def _(sp):
            sp.dma_start(out=qt_t[0], in_=qt_d[0]).then_inc(s_q0, 16)
            sp.dma_start(out=kp_t[0][:, 0:8],
                         in_=kp_d[0][:, 0:8]).then_inc(s_q0, 16)
            sp.dma_start(out=kp_t[0][:, 8:NPAIR],
                         in_=kp_d[0][:, 8:NPAIR]).then_inc(s_q0b, 16)
            sp.dma_start(out=qt_t[1], in_=qt_d[1]).then_inc(s_q1, 16)
            sp.dma_start(out=kp_t[1], in_=kp_d[1]).then_inc(s_q1, 16)
            sp.dma_start(out=mi_t, in_=mi_d).then_inc(s_mt, 16)
            for bh in range(NBH):
                sp.dma_start(out=mt_t[bh], in_=mt_d[bh]).then_inc(s_mt, 16)
            for rep in range(reps):
                for ib, sw in ((2, 1), (0, 2), (1, 6)):   # t5, t3, t4
                    sp.wait_ge(s_W, 6 * rep + sw)
                    sp.dma_start(out=w_d[1][:, ib, :],
                                 in_=W_t[1][:, ib, :]).then_inc(s_wsp, 16)
            sp.wait_ge(s_wsp, 48 * reps)
            sp.wait_ge(s_wact, 16 * reps)

        @block.scalar
        def _(a):
            a.wait_ge(s_q0, 32)
            for rep in range(reps):
                for p in range(F):
                    if rep >= 1:
                        a.wait_ge(s_pe, (rep - 1) * NUNIT + p + 1)
                    a.activation(T_f[:, p, :], qtrep(0), AF.Silu,
                                 bias=kp(0, p)).then_inc(s_S, 1)
                for bh, size, off, unit, ro_, _rw in GL:
                    a.wait_ge(s_T, rep * NPAIR_DVE +
                              (off + size - F if bh == 0
                               else (NPAIR - F) + off + size))
                    a.activation(T_t[:, ro_:ro_ + size, :],
                                 T_t[:, ro_:ro_ + size, :],
                                 AF.Silu).then_inc(s_S, 1)
                # ---- softmax (one table switch per rep) ----
                a.wait_ge(s_pm, 12 * rep + 6)
                if rep >= 1:
                    a.wait_ge(s_W, 6 * (rep - 1) + 6)   # E0 reuse
                a.activation(E_t[0], sc_t[0][:, :, 0:LK],
                             AF.Exp).then_inc(s_E, 1)
                a.wait_ge(s_pm, 12 * rep + 12)
                for t in (2, 0, 1):
                    a.activation(E_t[1][:, t, :], sc_t[1][:, t, 0:LK],
                                 AF.Exp,
                                 accum_out=sums_t[1][:, t:t + 1]
                                 ).then_inc(s_E, 1)
                # bh0's three tiles as ONE output DMA on ACT's queue
                a.wait_ge(s_W, 6 * rep + 5)
                a.dma_start(out=w_d[0], in_=W_t[0]).then_inc(s_wact, 16)

        @block.vector
        def _(v):
            v.wait_ge(s_q0, 32)
            for rep in range(reps):
                for gi, (bh, size, off, unit, ro_, rw) in enumerate(GL):
                    if bh == 1 and off == 0 and rep == 0:
                        v.wait_ge(s_q1, 32)
                    if rw is not None:
                        v.wait_ge(s_pe, rep * NUNIT + rw + 1)
                    elif rep >= 1:
                        v.wait_ge(s_pe, (rep - 1) * NUNIT + unit + 1)
                    for pl in range(size):
                        p = off + pl
                        if bh == 0 and rep == 0 and p == 8:
                            v.wait_ge(s_q0b, 16)
                        v.tensor_scalar_add(T_t[:, ro_ + pl, :], qtrep(bh),
                                            kp(bh, p)).then_inc(s_T, 1)
                # ---- softmax finalize ----
                v.wait_ge(s_E, 4 * rep + 1)
                if rep >= 1:
                    v.wait_ge(s_wact, 16 * rep)
                    v.wait_ge(s_wsp, 48 * rep)
                v.tensor_reduce(sums_t[0], E_t[0], mybir.AxisListType.X,
                                Alu.add)
                # s_W order: t5=1, t3=2, t0=3, t1=4, t2=5, t4=6
                v.wait_ge(s_E, 4 * rep + 2)           # acc5
                v.reciprocal(r_t[1][:, 2:3], sums_t[1][:, 2:3])
                v.wait_ge(s_E, 4 * rep + 3)           # acc3
                v.reciprocal(r_t[1][:, 0:1], sums_t[1][:, 0:1])
                v.drain()
                v.tensor_scalar_mul(W_t[1][:, 2, :], E_t[1][:, 2, :],
                                    r_t[1][:, 2:3]).then_inc(s_W, 1)
                v.tensor_scalar_mul(W_t[1][:, 0, :], E_t[1][:, 0, :],
                                    r_t[1][:, 0:1]).then_inc(s_W, 1)
                v.reciprocal(r_t[0], sums_t[0])
                v.wait_ge(s_E, 4 * rep + 4)           # acc4
                v.reciprocal(r_t[1][:, 1:2], sums_t[1][:, 1:2])
                v.drain()
                for t in range(NIB):
                    v.tensor_scalar_mul(W_t[0][:, t, :], E_t[0][:, t, :],
                                        r_t[0][:, t:t + 1]).then_inc(s_W, 1)
                v.tensor_scalar_mul(W_t[1][:, 1, :], E_t[1][:, 1, :],
                                    r_t[1][:, 1:2]).then_inc(s_W, 1)

        @block.tensor
        def _(t):
            t.wait_ge(s_q0, 32)
            t.wait_ge(s_mt, 48)
            for rep in range(reps):
                for bh in range(NBH):
                    if rep >= 1:
                        t.wait_ge(s_E, 4 * (rep - 1) + (1 if bh == 0 else 4))
                    if bh == 1 and rep == 0:
                        t.wait_ge(s_q1, 32)
                    units = [(u, s, o, r) for (b, s, o, u, r, _w) in GL
                             if b == bh]
                    if bh == 0:
                        units = [(u, 1, u, None) for u in range(F)] + units
                    for unit, size, off, ro_ in units:
                        t.wait_ge(s_S, rep * NUNIT + unit + 1)
                        ins = None
                        for pl in range(size):
                            p = off + pl
                            lhsT = (T_f[:, p, :] if unit < F
                                    else T_t[:, ro_ + pl, :])
                            for ib in range(NIB):
                                ins = nc.tensor.matmul(
                                    sc_t[bh][:, ib, 2 * p:2 * p + 2],
                                    lhsT[:, ib * 128:(ib + 1) * 128],
                                    a2(bh), start=(p == 0), stop=False)
                            if (2 * p + 2) % 64 == 0:
                                J = (2 * p + 2) // 64 - 1
                                base, jb = (64 * J) % 128, (64 * J) // 128
                                for ib in range(NIB):
                                    mm = nc.tensor.matmul(
                                        sc_t[bh][:, ib, 64 * J:64 * J + 64],
                                        mt_t[bh][base:base + 64, jb,
                                                 ib * 128:(ib + 1) * 128],
                                        mi_t[base:base + 64, :],
                                        start=False, stop=(J == 5))
                                mm.then_inc(s_pm, 1)
                        ins.then_inc(s_pe, 1)

    return nc


def _shard(q, k, a, mask):
    qf = q.reshape(B * H, LQ, D)
    kf = k.reshape(B * H, LK, D)
    mf = mask.reshape(B * H, LQ, LK)
    af = np.ascontiguousarray(
        np.broadcast_to(a.reshape(1, H, D), (B, H, D))).reshape(B * H, D)
    mi = np.zeros((128, 64), np.float32)
    mi[0:64, :] = np.eye(64) * -1e4
    mi[64:128, :] = np.eye(64) * -1e4
    mi = mi.astype(np.float16)
    in_maps = []
    for c in range(NCORES):
        sl = slice(NBH * c, NBH * (c + 1))
        qT = qf[sl].transpose(0, 2, 1)                              # [NBH,64,LQ]
        qt16 = np.zeros((NBH, 128, LQ + 2), np.float32)
        qt16[:, 0:64, 0:LQ] = qT
        qt16[:, 64:128, 0:LQ] = qT
        for j in range(NBH):
            qt16[j, 0:64, LQ] = af[NBH * c + j]
            qt16[j, 64:128, LQ + 1] = af[NBH * c + j]
        kp_ = kf[sl].reshape(NBH, NPAIR, 2, D).transpose(0, 2, 3, 1)
        kp_ = np.ascontiguousarray(kp_.reshape(NBH, 128, NPAIR),
                                   dtype=np.float32)
        # maskT[bh][j%128, j//128, i] = mask[bh][i, j]
        m = mf[sl].astype(np.float32)                               # [NBH,i,j]
        mt = m.transpose(0, 2, 1).reshape(NBH, NIB, 128, LQ)
        mt = np.ascontiguousarray(mt.transpose(0, 2, 1, 3))
        in_maps.append(dict(qt16=qt16.astype(np.float16), kp=kp_,
                            mt=mt.astype(np.float16), mi=mi))
    return in_maps


def kernel(q, k, attention, mask):
    global _built
    q = np.asarray(q, np.float32)
    k = np.asarray(k, np.float32)
    a = np.asarray(attention, np.float32)
    mask = np.asarray(mask).astype(bool)

    in_maps = _shard(q, k, a, mask)
    if _built is None:
        _built = _build()
    res = run_bass_kernel_spmd(_built, in_maps, core_ids=list(range(NCORES)))
    w = np.stack([res.results[c]["w"] for c in range(NCORES)], axis=0)
    # undo the [part, ib, j] device layout -> [i, j]
    w = w.transpose(0, 1, 3, 2, 4).reshape(NCORES, NBH, LQ, LK)
    return w.reshape(B, H, LQ, LK).astype(np.float32)


# revision 27
# speedup vs baseline: 1.0268x; 1.0131x over previous
"""GATv2 attention-weights kernel for 8 Trainium2 NeuronCores.

Problem (per full input):
    q: (2, 8, 384, 64) f32, k: (2, 8, 384, 64) f32,
    attention: (1, 8, 1, 1, 64) f32, mask: (2, 8, 384, 384) bool
    scores[b,h,i,j] = sum_d silu(q[b,h,i,d] + k[b,h,j,d]) * attention[h,d]
    out = softmax over j with mask (-inf before, 0 after)

Sharding: data-parallel over the 16 (b,h) pairs, 2 per core.

Per-core pipeline (raw bass, explicit semaphores; "jj,d" packing = two j
columns share the 128 partitions, d=64 each half):
  - ACT is the throughput floor (LQ*LK*D silu evals per (b,h) at 128
    lanes / 1.2 GHz, dtype-independent).  The first F pairs are computed
    directly on ACT via the fused activation bias operand
    silu(qT_rep + k_pair), removing the DVE round-trip from the ramp.
  - DVE builds the remaining T[(jj,d), i] = qT_rep + k_pair tiles in
    fp16 (4x DVE perf mode, 0.26 ns/elem) into a 192-pair fp16 ring;
    ACT computes silu in place per group.  Group sizes ramp so ACT
    never stalls, and are page-aligned to the ring (no wrap splits).
  - PE reduces over d with `a` folded into the weights:
    matmul(lhsT=T_pair fp16, rhs=a2 fp16) -> 2 score columns per pair,
    accumulated into per-(bh,i-block) PSUM banks in one long
    accumulation group (start on the first pair, stop on the last mask
    matmul).  The mask lands via extra fp16 matmuls (one per 64-column
    block: lhsT=maskT block, rhs=-1e4*I64), accumulating -1e4 into
    masked score positions (exp(-1e4) == 0), so there is no separate
    mask pass on any vector engine.
  - Softmax: a single activation-table switch to Exp after all silus
    (mid-stream Silu<->Exp switches are not supported by the backend).
    bh0's three tiles are one fused [128,3,384] exp; its row sums come
    from a DVE tensor_reduce that overlaps bh1's exps.  bh1's tiles use
    per-tile exps with accum_out so their sums are ready immediately.
    Reciprocal + scale on DVE; no per-row max (|scores| < 8).
  - Output DMAs alternate between the SP and ACT HWDGE queues in scale
    order (HWDGE setups and DMA transfers serialize device-wide, so the
    only lever is starting early and keeping the pipe busy).
"""

import numpy as np
from contextlib import ExitStack

import concourse.bass as bass
from concourse import mybir
from concourse.bass_utils import run_bass_kernel_spmd

B, H, LQ, LK, D = 2, 8, 384, 384, 64
NCORES = 8
NBH = (B * H) // NCORES        # 2 (b,h) pairs per core
NPAIR = LK // 2                # 192 j-pairs
NIB = LQ // 128                # 3 i-blocks
RING = 192                     # fp16 T ring capacity in pairs
F = 2                          # ACT-fused ramp pairs (bh0)

GROUPS0 = [3, 7, 16, 33, 67, 64]         # bh0 ramp (DVE 160 ns/pair)
GROUPS1 = [90, 90, 12]                   # bh1 (page-aligned, short PE tail)
assert sum(GROUPS0) == NPAIR - F and sum(GROUPS1) == NPAIR
NG0, NG1 = len(GROUPS0), len(GROUPS1)
NPAIR_DVE = (NPAIR - F) + NPAIR          # DVE-built pairs per rep
NUNIT = None                             # set in _build (after segmentation)

_f32 = mybir.dt.float32
_f16 = mybir.dt.float16

_built = None  # cache across calls


def _build(reps=1):
    global NUNIT
    AF = mybir.ActivationFunctionType
    Alu = mybir.AluOpType

    nc = bass.Bass("TRN2", target_bir_lowering=False, debug=False,
                   num_devices=NCORES)

    qt_d = nc.dram_tensor("qt16", [NBH, 128, LQ + 2 + F], _f16,
                          kind="ExternalInput").ap()
    kp_d = nc.dram_tensor("kp", [NBH, 128, NPAIR], _f32,
                          kind="ExternalInput").ap()
    mt_d = nc.dram_tensor("mt", [NBH, 128, NIB, LQ], _f16,
                          kind="ExternalInput").ap()
    mi_d = nc.dram_tensor("mi", [128, 64], _f16, kind="ExternalInput").ap()
    # w in "transposed" layout [part, ib, j], fp16 (host casts to f32;
    # w in [0,1] so fp16 adds only ~5e-4 rel err) -- halves output DMA
    w_d = nc.dram_tensor("w", [NBH, 128, NIB, LK], _f16,
                         kind="ExternalOutput").ap()

    qt_t = [nc.alloc_sbuf_tensor(f"qt{bh}", [128, LQ + 2 + F], _f16).ap()
            for bh in range(NBH)]
    kp_t = [nc.alloc_sbuf_tensor(f"kp{bh}", [128, NPAIR], _f32).ap()
            for bh in range(NBH)]
    mt_t = [nc.alloc_sbuf_tensor(f"mt{bh}", [128, NIB, LQ], _f16).ap()
            for bh in range(NBH)]
    mi_t = nc.alloc_sbuf_tensor("mi_t", [128, 64], _f16).ap()
    T_f = nc.alloc_sbuf_tensor("T_f", [128, F, LQ], _f16).ap()
    T_t = nc.alloc_sbuf_tensor("T_t", [128, RING, LQ], _f16).ap()
    E_t = [nc.alloc_sbuf_tensor(f"E{bh}", [128, NIB, LK], _f16).ap()
           for bh in range(NBH)]
    W_t = [nc.alloc_sbuf_tensor(f"W{bh}", [128, NIB, LK], _f16).ap()
           for bh in range(NBH)]
    sums_t = [nc.alloc_sbuf_tensor(f"sums{bh}", [128, NIB], _f32).ap()
              for bh in range(NBH)]
    r_t = [nc.alloc_sbuf_tensor(f"r{bh}", [128, NIB], _f32).ap()
           for bh in range(NBH)]
    sc_t = [nc.alloc_psum_tensor(f"sc{bh}", [128, NIB, 512], _f32).ap()
            for bh in range(NBH)]

    def qtrep(bh):
        return qt_t[bh][:, 0:LQ]

    def a2(bh):
        return qt_t[bh][:, LQ:LQ + 2]

    def kp(bh, p):
        return kp_t[bh][:, p:p + 1]

    # (bh, size, pair_offset, unit, ring_slot, reuse_wait_unit) per SEGMENT.
    # FIFO ring at pair granularity; groups crossing the ring end split.
    GL = []
    slot_owner = [None] * RING
    ro = 0

    def _alloc(bh, size, off):
        nonlocal ro
        while size > 0:
            seg = min(size, RING - ro)
            unit = len(GL) + F
            conflicts = [slot_owner[x] for x in range(ro, ro + seg)
                         if slot_owner[x] is not None]
            GL.append((bh, seg, off, unit, ro,
                       max(conflicts) if conflicts else None))
            for x in range(ro, ro + seg):
                slot_owner[x] = unit
            ro = (ro + seg) % RING
            off += seg
            size -= seg

    off = F
    for s in GROUPS0:
        _alloc(0, s, off)
        off += s
    if ro + GROUPS1[0] > RING:
        ro = 0          # skip waste slots so bh1 groups stay unsplit
    off = 0
    for s in GROUPS1:
        _alloc(1, s, off)
        off += s
    NUNIT = F + len(GL)

    with ExitStack() as ctx:
        s_q0 = ctx.enter_context(nc.semaphore("s_q0"))
        s_q1 = ctx.enter_context(nc.semaphore("s_q1"))
        s_q0b = ctx.enter_context(nc.semaphore("s_q0b"))
        s_mt = ctx.enter_context(nc.semaphore("s_mt"))
        s_T = ctx.enter_context(nc.semaphore("s_T"))
        s_S = ctx.enter_context(nc.semaphore("s_S"))
        s_pe = ctx.enter_context(nc.semaphore("s_pe"))
        s_pm = ctx.enter_context(nc.semaphore("s_pm"))
        s_E = ctx.enter_context(nc.semaphore("s_E"))
        s_W = ctx.enter_context(nc.semaphore("s_W"))
        s_wact = ctx.enter_context(nc.semaphore("s_wact"))
        s_wsp = ctx.enter_context(nc.semaphore("s_wsp"))
        block = ctx.enter_context(nc.Block())

        # scale emission order (drives s_W numbering): t3=1, t4=2, t0=3,
        # t1=4, t2=5, t5=6.  SP carries bh1 per-tile; ACT carries bh0 as
        # one merged DMA (w layout is [part, ib, j] so shapes line up).

        @block.sync
        def _(sp):
            sp.dma_start(out=qt_t[0], in_=qt_d[0]).then_inc(s_q0, 16)
            sp.dma_start(out=kp_t[0][:, 0:8],
                         in_=kp_d[0][:, 0:8]).then_inc(s_q0b, 16)
            sp.dma_start(out=kp_t[0][:, 8:NPAIR],
                         in_=kp_d[0][:, 8:NPAIR]).then_inc(s_q0b, 16)
            sp.dma_start(out=qt_t[1], in_=qt_d[1]).then_inc(s_q1, 16)
            sp.dma_start(out=kp_t[1], in_=kp_d[1]).then_inc(s_q1, 16)
            sp.dma_start(out=mi_t, in_=mi_d).then_inc(s_mt, 16)
            for bh in range(NBH):
                sp.dma_start(out=mt_t[bh], in_=mt_d[bh]).then_inc(s_mt, 16)
            for rep in range(reps):
                for ib, sw in ((2, 1), (0, 2), (1, 6)):   # t5, t3, t4
                    sp.wait_ge(s_W, 6 * rep + sw)
                    sp.dma_start(out=w_d[1][:, ib, :],
                                 in_=W_t[1][:, ib, :]).then_inc(s_wsp, 16)
            sp.wait_ge(s_wsp, 48 * reps)
            sp.wait_ge(s_wact, 16 * reps)

        @block.scalar
        def _(a):
            a.wait_ge(s_q0, 16)
            for rep in range(reps):
                for p in range(F):
                    if rep >= 1:
                        a.wait_ge(s_pe, (rep - 1) * NUNIT + p + 1)
                    a.activation(T_f[:, p, :], qtrep(0), AF.Silu,
                                 bias=qt_t[0][:, LQ + 2 + p:LQ + 3 + p]
                                 ).then_inc(s_S, 1)
                for bh, size, off, unit, ro_, _rw in GL:
                    a.wait_ge(s_T, rep * NPAIR_DVE +
                              (off + size - F if bh == 0
                               else (NPAIR - F) + off + size))
                    a.activation(T_t[:, ro_:ro_ + size, :],
                                 T_t[:, ro_:ro_ + size, :],
                                 AF.Silu).then_inc(s_S, 1)
                # ---- softmax (one table switch per rep) ----
                a.wait_ge(s_pm, 12 * rep + 6)
                if rep >= 1:
                    a.wait_ge(s_W, 6 * (rep - 1) + 6)   # E0 reuse
                a.activation(E_t[0], sc_t[0][:, :, 0:LK],
                             AF.Exp).then_inc(s_E, 1)
                a.wait_ge(s_pm, 12 * rep + 12)
                for t in (2, 0, 1):
                    a.activation(E_t[1][:, t, :], sc_t[1][:, t, 0:LK],
                                 AF.Exp,
                                 accum_out=sums_t[1][:, t:t + 1]
                                 ).then_inc(s_E, 1)
                # bh0's three tiles as ONE output DMA on ACT's queue
                a.wait_ge(s_W, 6 * rep + 5)
                a.dma_start(out=w_d[0], in_=W_t[0]).then_inc(s_wact, 16)

        @block.vector
        def _(v):
            v.wait_ge(s_q0, 16)
            for rep in range(reps):
                for gi, (bh, size, off, unit, ro_, rw) in enumerate(GL):
                    if bh == 1 and off == 0 and rep == 0:
                        v.wait_ge(s_q1, 32)
                    if rw is not None:
                        v.wait_ge(s_pe, rep * NUNIT + rw + 1)
                    elif rep >= 1:
                        v.wait_ge(s_pe, (rep - 1) * NUNIT + unit + 1)
                    for pl in range(size):
                        p = off + pl
                        if bh == 0 and rep == 0 and p == F:
                            v.wait_ge(s_q0b, 16)
                        if bh == 0 and rep == 0 and p == 8:
                            v.wait_ge(s_q0b, 32)
                        v.tensor_scalar_add(T_t[:, ro_ + pl, :], qtrep(bh),
                                            kp(bh, p)).then_inc(s_T, 1)
                # ---- softmax finalize ----
                v.wait_ge(s_E, 4 * rep + 1)
                if rep >= 1:
                    v.wait_ge(s_wact, 16 * rep)
                    v.wait_ge(s_wsp, 48 * rep)
                v.tensor_reduce(sums_t[0], E_t[0], mybir.AxisListType.X,
                                Alu.add)
                # s_W order: t5=1, t3=2, t0=3, t1=4, t2=5, t4=6
                v.wait_ge(s_E, 4 * rep + 2)           # acc5
                v.reciprocal(r_t[1][:, 2:3], sums_t[1][:, 2:3])
                v.wait_ge(s_E, 4 * rep + 3)           # acc3
                v.reciprocal(r_t[1][:, 0:1], sums_t[1][:, 0:1])
                v.drain()
                v.tensor_scalar_mul(W_t[1][:, 2, :], E_t[1][:, 2, :],
                                    r_t[1][:, 2:3]).then_inc(s_W, 1)
                v.tensor_scalar_mul(W_t[1][:, 0, :], E_t[1][:, 0, :],
                                    r_t[1][:, 0:1]).then_inc(s_W, 1)
                v.reciprocal(r_t[0], sums_t[0])
                v.wait_ge(s_E, 4 * rep + 4)           # acc4
                v.reciprocal(r_t[1][:, 1:2], sums_t[1][:, 1:2])
                v.drain()
                for t in range(NIB):
                    v.tensor_scalar_mul(W_t[0][:, t, :], E_t[0][:, t, :],
                                        r_t[0][:, t:t + 1]).then_inc(s_W, 1)
                v.tensor_scalar_mul(W_t[1][:, 1, :], E_t[1][:, 1, :],
                                    r_t[1][:, 1:2]).then_inc(s_W, 1)

        @block.tensor
        def _(t):
            t.wait_ge(s_q0, 16)
            t.wait_ge(s_mt, 48)
            for rep in range(reps):
                for bh in range(NBH):
                    if rep >= 1:
                        t.wait_ge(s_E, 4 * (rep - 1) + (1 if bh == 0 else 4))
                    if bh == 1 and rep == 0:
                        t.wait_ge(s_q1, 32)
                    units = [(u, s, o, r) for (b, s, o, u, r, _w) in GL
                             if b == bh]
                    if bh == 0:
                        units = [(u, 1, u, None) for u in range(F)] + units
                    for unit, size, off, ro_ in units:
                        t.wait_ge(s_S, rep * NUNIT + unit + 1)
                        ins = None
                        for pl in range(size):
                            p = off + pl
                            lhsT = (T_f[:, p, :] if unit < F
                                    else T_t[:, ro_ + pl, :])
                            for ib in range(NIB):
                                ins = nc.tensor.matmul(
                                    sc_t[bh][:, ib, 2 * p:2 * p + 2],
                                    lhsT[:, ib * 128:(ib + 1) * 128],
                                    a2(bh), start=(p == 0), stop=False)
                            if (2 * p + 2) % 64 == 0:
                                J = (2 * p + 2) // 64 - 1
                                base, jb = (64 * J) % 128, (64 * J) // 128
                                for ib in range(NIB):
                                    mm = nc.tensor.matmul(
                                        sc_t[bh][:, ib, 64 * J:64 * J + 64],
                                        mt_t[bh][base:base + 64, jb,
                                                 ib * 128:(ib + 1) * 128],
                                        mi_t[base:base + 64, :],
                                        start=False, stop=(J == 5))
                                mm.then_inc(s_pm, 1)
                        ins.then_inc(s_pe, 1)

    return nc


def _shard(q, k, a, mask):
    qf = q.reshape(B * H, LQ, D)
    kf = k.reshape(B * H, LK, D)
    mf = mask.reshape(B * H, LQ, LK)
    af = np.ascontiguousarray(
        np.broadcast_to(a.reshape(1, H, D), (B, H, D))).reshape(B * H, D)
    mi = np.zeros((128, 64), np.float32)
    mi[0:64, :] = np.eye(64) * -1e4
    mi[64:128, :] = np.eye(64) * -1e4
    mi = mi.astype(np.float16)
    in_maps = []
    for c in range(NCORES):
        sl = slice(NBH * c, NBH * (c + 1))
        qT = qf[sl].transpose(0, 2, 1)                              # [NBH,64,LQ]
        qt16 = np.zeros((NBH, 128, LQ + 2 + F), np.float32)
        qt16[:, 0:64, 0:LQ] = qT
        qt16[:, 64:128, 0:LQ] = qT
        for j in range(NBH):
            qt16[j, 0:64, LQ] = af[NBH * c + j]
            qt16[j, 64:128, LQ + 1] = af[NBH * c + j]
        kp_ = kf[sl].reshape(NBH, NPAIR, 2, D).transpose(0, 2, 3, 1)
        kp_ = np.ascontiguousarray(kp_.reshape(NBH, 128, NPAIR),
                                   dtype=np.float32)
        qt16[:, :, LQ + 2:LQ + 2 + F] = kp_[:, :, 0:F]
        # maskT[bh][j%128, j//128, i] = mask[bh][i, j]
        m = mf[sl].astype(np.float32)                               # [NBH,i,j]
        mt = m.transpose(0, 2, 1).reshape(NBH, NIB, 128, LQ)
        mt = np.ascontiguousarray(mt.transpose(0, 2, 1, 3))
        in_maps.append(dict(qt16=qt16.astype(np.float16), kp=kp_,
                            mt=mt.astype(np.float16), mi=mi))
    return in_maps


def kernel(q, k, attention, mask):
    global _built
    q = np.asarray(q, np.float32)
    k = np.asarray(k, np.float32)
    a = np.asarray(attention, np.float32)
    mask = np.asarray(mask).astype(bool)

    in_maps = _shard(q, k, a, mask)
    if _built is None:
        _built = _build()
    res = run_bass_kernel_spmd(_built, in_maps, core_ids=list(range(NCORES)))
    w = np.stack([res.results[c]["w"] for c in range(NCORES)], axis=0)
    # undo the [part, ib, j] device layout -> [i, j]
    w = w.transpose(0, 1, 3, 2, 4).reshape(NCORES, NBH, LQ, LK)
    return w.reshape(B, H, LQ, LK).astype(np.float32)
